# revision 2
# baseline (speedup 1.0000x reference)
"""RNN-T Joint network kernel for Trainium2 (Bass/Tile), 8-core data-parallel.

Math (per batch b):
  hf = f[b] @ W1[:1024]            # (T=256, J=640)
  hg = g[b] @ W1[1024:]            # (U=65,  J=640)
  h[t,u,:]   = relu(hf[t] + hg[u] + b1)
  out[t,u,:] = h[t,u,:] @ W2 + b2  # (256, 65, 1024)

Sharding: data-parallel over B=8, one utterance per core.  Host-side prep
(part of the sharding step): f/g are transposed and cast to bf16, W1 is
re-laid-out into j-chunk-major bf16 blocks, W2 cast to bf16 — so the device
program does no transposes/casts and the DMA critical path at startup is
minimal.

Device layout (per core, u-major):
  - hfT[j, t] (f32) and hgT'[j, u] = hgT + b1 (f32) resident in SBUF
    (j on partitions), computed by PE from the pre-transposed inputs.
  - For each u: H_u[j, t] = relu(hfT[j, t] + hgT'[j, u]) built by ScalarE
    (bias = per-partition column hgT'[:, u]), cast to bf16.
  - PE: out_tile[t128, v512] += H_u[jc][:, t128].T @ W2[jc][:, v512],
    5 j-chunks accumulated in PSUM (fp32).
  - VectorE drains PSUM + adds broadcast b2, DMA straight to HBM.
"""

import numpy as np

T, U = 256, 65
EH, PH, J, V = 1024, 320, 640, 1024
JC = J // 128           # 5 j-chunks
HC = EH // 128          # 8 h-chunks
N_CORES = 8

_CACHE = {}


def _build_nc():
    import concourse.bass as bass
    import concourse.bacc as bacc
    import concourse.mybir as mybir
    from concourse import tile

    f32 = mybir.dt.float32
    bf16 = mybir.dt.bfloat16
    Relu = mybir.ActivationFunctionType.Relu
    add = mybir.AluOpType.add

    nc = bacc.Bacc(None, target_bir_lowering=False)

    fT_d = nc.declare_dram_parameter("fT", [EH, T], bf16, isOutput=False)
    gT_d = nc.declare_dram_parameter("gT", [PH, U], bf16, isOutput=False)
    # W1f/W1g j-chunk-major: [JC, rows, 128] so each (c, h) tile is contiguous
    W1f_d = nc.declare_dram_parameter("W1f", [JC, EH, 128], bf16, isOutput=False)
    W1g_d = nc.declare_dram_parameter("W1g", [JC, PH, 128], bf16, isOutput=False)
    W2_d = nc.declare_dram_parameter("W2", [J, V], bf16, isOutput=False)
    b1_d = nc.declare_dram_parameter("b1", [J], f32, isOutput=False)
    b2_d = nc.declare_dram_parameter("b2", [V], f32, isOutput=False)
    out_d = nc.declare_dram_parameter("out", [T, U, V], f32, isOutput=True)

    # W1g partition chunks (PH = 320 = 128 + 128 + 64)
    g_chunks = [(0, 128), (128, 128), (256, 64)]

    with tile.TileContext(nc) as tc:
        with tc.tile_pool(name="const", bufs=1) as cpool:
            # Preload the ScalarE activation table (Relu) off the critical
            # path: first act instruction pays ~1.3us table load otherwise.
            dumin = cpool.tile([128, 1], f32)
            nc.vector.memset(dumin[:], 0.0)
            dumout = cpool.tile([128, 1], f32)
            nc.scalar.activation(dumout[:], dumin[:], Relu, bias=0.0, scale=1.0)

            # ---------------- DMA (priority order) ----------------
            # critical chain first: fT + W1f[c0] feed the first hfT matmuls
            fTb = []
            for h in range(HC):
                t = cpool.tile([128, T], bf16, tag=f"fT{h}")
                nc.sync.dma_start(out=t[:], in_=fT_d[h * 128:(h + 1) * 128, :])
                fTb.append(t)
            W1fb = [[None] * HC for _ in range(JC)]
            for h in range(HC):
                t = cpool.tile([128, 128], bf16, tag=f"w1f0{h}")
                nc.sync.dma_start(out=t[:], in_=W1f_d[0, h * 128:(h + 1) * 128, :])
                W1fb[0][h] = t

            # small inputs for hgT and biases
            gTb = []
            for pc, (po, pn) in enumerate(g_chunks):
                t = cpool.tile([pn, U], bf16, tag=f"gT{pc}")
                nc.sync.dma_start(out=t[:], in_=gT_d[po:po + pn, :])
                gTb.append(t)
            W1gb = [[None] * 3 for _ in range(JC)]
            for c in range(JC):
                for pc, (po, pn) in enumerate(g_chunks):
                    t = cpool.tile([pn, 128], bf16, tag=f"w1g{c}{pc}")
                    nc.sync.dma_start(out=t[:], in_=W1g_d[c, po:po + pn, :])
                    W1gb[c][pc] = t
            b1sb = cpool.tile([128, JC], f32)
            nc.sync.dma_start(out=b1sb[:], in_=b1_d[:].rearrange("(c p) -> p c", p=128))
            b2row = cpool.tile([1, V], f32)
            nc.sync.dma_start(out=b2row[:], in_=b2_d[:].rearrange("(a v) -> a v", a=1))

            # W2 chunk 0 (needed by first main matmul), then the rest of the
            # weights interleaved so chunk c arrives before it's consumed.
            W2b = [None] * JC
            t = cpool.tile([128, V], bf16, tag="w2b0")
            nc.sync.dma_start(out=t[:], in_=W2_d[0:128, :])
            W2b[0] = t
            for c in range(1, JC):
                for h in range(HC):
                    t = cpool.tile([128, 128], bf16, tag=f"w1f{c}{h}")
                    nc.sync.dma_start(out=t[:],
                                      in_=W1f_d[c, h * 128:(h + 1) * 128, :])
                    W1fb[c][h] = t
                t = cpool.tile([128, V], bf16, tag=f"w2b{c}")
                nc.sync.dma_start(out=t[:], in_=W2_d[c * 128:(c + 1) * 128, :])
                W2b[c] = t

            # ---------------- prologue: first layer on PE ----------------
            hfTs = []     # f32 [128, T] x JC   (hf^T)
            hgTs = []     # f32 [128, U] x JC   (hg^T + b1)

            with (
                tc.tile_pool(name="ppsumA", bufs=2, space=bass.MemorySpace.PSUM) as ppA,
                tc.tile_pool(name="ppsumB", bufs=2, space=bass.MemorySpace.PSUM) as ppB,
            ):
                # hf^T[c] = sum_h W1f[c][h].T @ fT[h]
                def hfT_chunk(c):
                    pf = ppA.tile([128, T], f32, tag="pf")
                    for h in range(HC):
                        nc.tensor.matmul(pf[:], W1fb[c][h][:], fTb[h][:],
                                         start=(h == 0), stop=(h == HC - 1))
                    t = cpool.tile([128, T], f32, tag=f"hfT{c}")
                    nc.vector.tensor_copy(t[:], pf[:])
                    hfTs.append(t)

                hfT_chunk(0)

                # hg^T[c] = sum_pc W1g[c][pc].T @ gT[pc]  (+ b1 on drain)
                for c in range(JC):
                    ph = ppB.tile([128, U], f32, tag="ph")
                    for pc in range(3):
                        nc.tensor.matmul(ph[:], W1gb[c][pc][:], gTb[pc][:],
                                         start=(pc == 0), stop=(pc == 2))
                    t = cpool.tile([128, U], f32, tag=f"hgT{c}")
                    nc.vector.tensor_scalar(t[:], ph[:], b1sb[:, c:c + 1], None, add)
                    hgTs.append(t)

                # broadcast b2 across 128 partitions via rank-1 matmul
                ones = cpool.tile([1, 128], f32)
                nc.vector.memset(ones[:], 1.0)
                b2bc = cpool.tile([128, V], f32)
                for vh in range(2):
                    pb = ppB.tile([128, 512], f32, tag="pb")
                    nc.tensor.matmul(pb[:], ones[:], b2row[:, vh * 512:(vh + 1) * 512],
                                     start=True, stop=True)
                    nc.vector.tensor_copy(b2bc[:, vh * 512:(vh + 1) * 512], pb[:])

                for c in range(1, JC):
                    hfT_chunk(c)

            # ---------------- main loop over u ----------------
            with (
                tc.tile_pool(name="hpool", bufs=4) as hpool,
                tc.tile_pool(name="opool", bufs=4) as opool,
                tc.tile_pool(name="mpsum", bufs=2, space=bass.MemorySpace.PSUM) as mpsum,
            ):
                for u in range(U):
                    Hs = []
                    for c in range(JC):
                        ht = hpool.tile([128, T], bf16, tag=f"H{c}")
                        nc.scalar.activation(ht[:], hfTs[c][:], Relu,
                                             bias=hgTs[c][:, u:u + 1], scale=1.0)
                        Hs.append(ht)
                    for tt in range(2):
                        ps0 = mpsum.tile([128, 512], f32, tag=f"ps{tt}0")
                        ps1 = mpsum.tile([128, 512], f32, tag=f"ps{tt}1")
                        ps = [ps0, ps1]
                        for c in range(JC):
                            lhsT = Hs[c][:, tt * 128:(tt + 1) * 128]
                            nc.tensor.matmul(ps[0][:], lhsT, W2b[c][:, 0:512],
                                             start=(c == 0), stop=(c == JC - 1))
                            nc.tensor.matmul(ps[1][:], lhsT, W2b[c][:, 512:1024],
                                             start=(c == 0), stop=(c == JC - 1))
                        for vh in range(2):
                            ot = opool.tile([128, 512], f32, tag=f"o{tt}{vh}")
                            nc.vector.tensor_tensor(
                                ot[:], ps[vh][:],
                                b2bc[:, vh * 512:(vh + 1) * 512], add)
                            nc.sync.dma_start(
                                out=out_d[tt * 128:(tt + 1) * 128, u,
                                          vh * 512:(vh + 1) * 512],
                                in_=ot[:])
    nc.compile()
    return nc


def _get_nc():
    if "nc" not in _CACHE:
        _CACHE["nc"] = _build_nc()
    return _CACHE["nc"]


def _prep_core_inputs(f_b, g_b, W1fc, W1gc, W2b, b1, b2):
    import ml_dtypes

    bf16 = ml_dtypes.bfloat16
    return {
        "fT": np.ascontiguousarray(f_b.T.astype(bf16)),
        "gT": np.ascontiguousarray(g_b.T.astype(bf16)),
        "W1f": W1fc,
        "W1g": W1gc,
        "W2": W2b,
        "b1": b1,
        "b2": b2,
    }


def run(f, g, W1, b1, W2, b2, trace=False):
    """Returns (full_output, BassKernelResults)."""
    import ml_dtypes
    from concourse.bass_utils import run_bass_kernel_spmd

    bf16 = ml_dtypes.bfloat16
    nc = _get_nc()

    W1 = np.asarray(W1, dtype=np.float32)
    # j-chunk-major re-layout of W1 (shared across cores)
    W1fc = np.ascontiguousarray(
        np.stack([W1[:EH, c * 128:(c + 1) * 128] for c in range(JC)])).astype(bf16)
    W1gc = np.ascontiguousarray(
        np.stack([W1[EH:, c * 128:(c + 1) * 128] for c in range(JC)])).astype(bf16)
    W2b = np.ascontiguousarray(np.asarray(W2, dtype=np.float32)).astype(bf16)
    b1 = np.ascontiguousarray(np.asarray(b1, dtype=np.float32))
    b2 = np.ascontiguousarray(np.asarray(b2, dtype=np.float32))
    f = np.asarray(f, dtype=np.float32)
    g = np.asarray(g, dtype=np.float32)

    in_maps = [
        _prep_core_inputs(f[i], g[i], W1fc, W1gc, W2b, b1, b2)
        for i in range(N_CORES)
    ]
    res = run_bass_kernel_spmd(nc, in_maps, list(range(N_CORES)), trace=trace)
    out = np.stack([res.results[i]["out"] for i in range(N_CORES)], axis=0)
    return out, res


def kernel(f, g, W1, b1, W2, b2):
    out, _ = run(f, g, W1, b1, W2, b2)
    return out


# revision 3
# speedup vs baseline: 1.1121x; 1.1121x over previous
"""RNN-T Joint network kernel for Trainium2 (Bass/Tile), 8-core data-parallel.

Math (per batch b):
  hf = f[b] @ W1[:1024]            # (T=256, J=640)
  hg = g[b] @ W1[1024:]            # (U=65,  J=640)
  h[t,u,:]   = relu(hf[t] + hg[u] + b1)
  out[t,u,:] = h[t,u,:] @ W2 + b2  # (256, 65, 1024)

Sharding: data-parallel over B=8, one utterance per core.  Host-side prep
(part of the sharding step): all inputs are cast to bf16 and packed into
partition-major layouts so every input is a single large DMA with fully
contiguous >=2KB per-partition lines; the device program does no
transposes or dtype casts.

Device schedule (per core, u-major):
  - hfT[j, t] (f32) and hgT'[j, u] = hgT + b1 (f32) resident in SBUF
    (j on partitions), computed by PE from the pre-transposed inputs.
  - For each u: H_u[j, t] = relu(hfT[j, t] + hgT'[j, u]) built by ScalarE
    (bias = per-partition column hgT'[:, u]), cast to bf16.
  - PE: out_tile[t128, v512] += H_u[jc][:, t128].T @ W2[jc][:, v512],
    5 j-chunks accumulated in PSUM (fp32).
  - VectorE drains PSUM + adds broadcast b2 into a [128, 1024] tile,
    one DMA per (u, t-half) straight to HBM.
"""

import numpy as np

T, U = 256, 65
EH, PH, J, V = 1024, 320, 640, 1024
JC = J // 128           # 5 j-chunks
HC = EH // 128          # 8 h-chunks (f side)
GC = 3                  # g-side chunks (PH padded 320 -> 384 = 3*128)
N_CORES = 8

_CACHE = {}


def _build_nc():
    import concourse.bass as bass
    import concourse.bacc as bacc
    import concourse.mybir as mybir
    from concourse import tile

    f32 = mybir.dt.float32
    bf16 = mybir.dt.bfloat16
    Relu = mybir.ActivationFunctionType.Relu
    add = mybir.AluOpType.add

    nc = bacc.Bacc(None, target_bir_lowering=False)

    # packed, partition-major inputs (see _pack_* helpers)
    fT_d = nc.declare_dram_parameter("fTp", [128, HC * T], bf16, isOutput=False)
    gT_d = nc.declare_dram_parameter("gTp", [128, GC * U], bf16, isOutput=False)
    W1f_d = nc.declare_dram_parameter("W1fp", [128, JC * HC * 128], bf16,
                                      isOutput=False)
    W1g_d = nc.declare_dram_parameter("W1gp", [128, JC * GC * 128], bf16,
                                      isOutput=False)
    W2_d = nc.declare_dram_parameter("W2p", [128, JC * V], bf16, isOutput=False)
    b1_d = nc.declare_dram_parameter("b1p", [128, JC], f32, isOutput=False)
    b2_d = nc.declare_dram_parameter("b2p", [1, V], f32, isOutput=False)
    out_d = nc.declare_dram_parameter("out", [T, U, V], f32, isOutput=True)

    with tile.TileContext(nc) as tc:
        with tc.tile_pool(name="const", bufs=1) as cpool:
            # Preload the ScalarE activation table (Relu) off the critical
            # path: the first act instruction pays ~1.3us table load.
            dumin = cpool.tile([128, 1], f32)
            nc.vector.memset(dumin[:], 0.0)
            dumout = cpool.tile([128, 1], f32)
            nc.scalar.activation(dumout[:], dumin[:], Relu, bias=0.0, scale=1.0)

            # ---------------- DMA (priority order) ----------------
            # Critical chain first, split in halves so the first hfT matmuls
            # can start while the second half streams in.
            fTall = cpool.tile([128, HC * T], bf16)
            W1fall = cpool.tile([128, JC * HC * 128], bf16)
            half_f = (HC // 2) * T
            half_w = (HC // 2) * 128
            nc.sync.dma_start(out=fTall[:, :half_f], in_=fT_d[:, :half_f])
            nc.sync.dma_start(out=W1fall[:, :half_w], in_=W1f_d[:, :half_w])
            nc.sync.dma_start(out=fTall[:, half_f:], in_=fT_d[:, half_f:])
            nc.sync.dma_start(out=W1fall[:, half_w:HC * 128],
                              in_=W1f_d[:, half_w:HC * 128])

            # small inputs for hgT, biases
            gTall = cpool.tile([128, GC * U], bf16)
            nc.sync.dma_start(out=gTall[:], in_=gT_d[:])
            W1gall = cpool.tile([128, JC * GC * 128], bf16)
            nc.sync.dma_start(out=W1gall[:], in_=W1g_d[:])
            b1sb = cpool.tile([128, JC], f32)
            nc.sync.dma_start(out=b1sb[:], in_=b1_d[:])
            b2row = cpool.tile([1, V], f32)
            nc.sync.dma_start(out=b2row[:], in_=b2_d[:])

            # W2 chunk 0 (needed by the first main matmul), then remaining
            # W1f chunks interleaved with remaining W2 chunks.
            W2all = cpool.tile([128, JC * V], bf16)
            nc.sync.dma_start(out=W2all[:, :V], in_=W2_d[:, :V])
            for c in range(1, JC):
                nc.sync.dma_start(
                    out=W1fall[:, c * HC * 128:(c + 1) * HC * 128],
                    in_=W1f_d[:, c * HC * 128:(c + 1) * HC * 128])
                nc.sync.dma_start(out=W2all[:, c * V:(c + 1) * V],
                                  in_=W2_d[:, c * V:(c + 1) * V])

            def fT(h):
                return fTall[:, h * T:(h + 1) * T]

            def w1f(c, h):
                o = (c * HC + h) * 128
                return W1fall[:, o:o + 128]

            def gT(pc):
                return gTall[:, pc * U:(pc + 1) * U]

            def w1g(c, pc):
                o = (c * GC + pc) * 128
                return W1gall[:, o:o + 128]

            def w2(c, lo, hi):
                return W2all[:, c * V + lo:c * V + hi]

            # ---------------- prologue: first layer on PE ----------------
            hfTs = [None] * JC   # f32 [128, T]  (hf^T)
            hgTs = []            # f32 [128, U]  (hg^T + b1)

            with (
                tc.tile_pool(name="ppsumA", bufs=2, space=bass.MemorySpace.PSUM) as ppA,
                tc.tile_pool(name="ppsumB", bufs=2, space=bass.MemorySpace.PSUM) as ppB,
            ):
                def hfT_chunk(c):
                    pf = ppA.tile([128, T], f32, tag="pf")
                    for h in range(HC):
                        nc.tensor.matmul(pf[:], w1f(c, h), fT(h),
                                         start=(h == 0), stop=(h == HC - 1))
                    t = cpool.tile([128, T], f32, tag=f"hfT{c}")
                    nc.vector.tensor_copy(t[:], pf[:])
                    hfTs[c] = t

                hfT_chunk(0)

                # hg^T[c] = sum_pc W1g[c][pc].T @ gT[pc]  (+ b1 on drain)
                for c in range(JC):
                    ph = ppB.tile([128, U], f32, tag="ph")
                    for pc in range(GC):
                        nc.tensor.matmul(ph[:], w1g(c, pc), gT(pc),
                                         start=(pc == 0), stop=(pc == GC - 1))
                    t = cpool.tile([128, U], f32, tag=f"hgT{c}")
                    nc.vector.tensor_scalar(t[:], ph[:], b1sb[:, c:c + 1], None, add)
                    hgTs.append(t)

                # broadcast b2 across 128 partitions via rank-1 matmul
                ones = cpool.tile([1, 128], f32)
                nc.vector.memset(ones[:], 1.0)
                b2bc = cpool.tile([128, V], f32)
                for vh in range(2):
                    pb = ppB.tile([128, 512], f32, tag="pb")
                    nc.tensor.matmul(pb[:], ones[:], b2row[:, vh * 512:(vh + 1) * 512],
                                     start=True, stop=True)
                    nc.vector.tensor_copy(b2bc[:, vh * 512:(vh + 1) * 512], pb[:])

                for c in range(1, JC):
                    hfT_chunk(c)

            # ---------------- main loop over u ----------------
            with (
                tc.tile_pool(name="hpool", bufs=4) as hpool,
                tc.tile_pool(name="opool", bufs=3) as opool,
                tc.tile_pool(name="mpsum", bufs=2, space=bass.MemorySpace.PSUM) as mpsum,
            ):
                for u in range(U):
                    Hs = []
                    for c in range(JC):
                        ht = hpool.tile([128, T], bf16, tag=f"H{c}")
                        nc.scalar.activation(ht[:], hfTs[c][:], Relu,
                                             bias=hgTs[c][:, u:u + 1], scale=1.0)
                        Hs.append(ht)
                    for tt in range(2):
                        ps0 = mpsum.tile([128, 512], f32, tag=f"ps{tt}0")
                        ps1 = mpsum.tile([128, 512], f32, tag=f"ps{tt}1")
                        ps = [ps0, ps1]
                        for c in range(JC):
                            lhsT = Hs[c][:, tt * 128:(tt + 1) * 128]
                            nc.tensor.matmul(ps[0][:], lhsT, w2(c, 0, 512),
                                             start=(c == 0), stop=(c == JC - 1))
                            nc.tensor.matmul(ps[1][:], lhsT, w2(c, 512, 1024),
                                             start=(c == 0), stop=(c == JC - 1))
                        ot = opool.tile([128, V], f32, tag=f"o{tt}")
                        for vh in range(2):
                            nc.vector.tensor_tensor(
                                ot[:, vh * 512:(vh + 1) * 512], ps[vh][:],
                                b2bc[:, vh * 512:(vh + 1) * 512], add)
                        nc.sync.dma_start(
                            out=out_d[tt * 128:(tt + 1) * 128, u, :],
                            in_=ot[:])
    nc.compile()
    return nc


def _get_nc():
    if "nc" not in _CACHE:
        _CACHE["nc"] = _build_nc()
    return _CACHE["nc"]


def _pack_shared(W1, b1, W2):
    """Partition-major packed weights, shared across cores (bf16)."""
    import ml_dtypes

    bf16 = ml_dtypes.bfloat16
    W1 = np.asarray(W1, dtype=np.float32)
    W2 = np.asarray(W2, dtype=np.float32)
    # W1f: [p, c, h, k] with source index [h*128+p, c*128+k]
    W1fp = np.ascontiguousarray(
        W1[:EH].reshape(HC, 128, JC, 128).transpose(1, 2, 0, 3)
        .reshape(128, JC * HC * 128)).astype(bf16)
    # W1g: pad rows to 384, then [p, c, pc, k]
    W1g = np.zeros((GC * 128, J), dtype=np.float32)
    W1g[:PH] = W1[EH:]
    W1gp = np.ascontiguousarray(
        W1g.reshape(GC, 128, JC, 128).transpose(1, 2, 0, 3)
        .reshape(128, JC * GC * 128)).astype(bf16)
    # W2: [p, c, v] with source [c*128+p, v]
    W2p = np.ascontiguousarray(
        W2.reshape(JC, 128, V).transpose(1, 0, 2).reshape(128, JC * V)
    ).astype(bf16)
    b1p = np.ascontiguousarray(
        np.asarray(b1, dtype=np.float32).reshape(JC, 128).T)
    return W1fp, W1gp, W2p, b1p


def _pack_core(f_b, g_b):
    import ml_dtypes

    bf16 = ml_dtypes.bfloat16
    # fT packed: [p, h, t] with source f[t, h*128+p]
    fTp = np.ascontiguousarray(
        f_b.T.reshape(HC, 128, T).transpose(1, 0, 2).reshape(128, HC * T)
    ).astype(bf16)
    # gT packed: pad rows of g^T [PH, U] to 384 = GC*128
    gTfull = np.zeros((GC * 128, U), dtype=np.float32)
    gTfull[:PH] = g_b.T
    gTp = np.ascontiguousarray(
        gTfull.reshape(GC, 128, U).transpose(1, 0, 2).reshape(128, GC * U)
    ).astype(bf16)
    return fTp, gTp


def run(f, g, W1, b1, W2, b2, trace=False):
    """Returns (full_output, BassKernelResults)."""
    from concourse.bass_utils import run_bass_kernel_spmd

    nc = _get_nc()

    W1fp, W1gp, W2p, b1p = _pack_shared(W1, b1, W2)
    b2p = np.ascontiguousarray(np.asarray(b2, dtype=np.float32).reshape(1, V))
    f = np.asarray(f, dtype=np.float32)
    g = np.asarray(g, dtype=np.float32)

    in_maps = []
    for i in range(N_CORES):
        fTp, gTp = _pack_core(f[i], g[i])
        in_maps.append({
            "fTp": fTp,
            "gTp": gTp,
            "W1fp": W1fp,
            "W1gp": W1gp,
            "W2p": W2p,
            "b1p": b1p,
            "b2p": b2p,
        })
    res = run_bass_kernel_spmd(nc, in_maps, list(range(N_CORES)), trace=trace)
    out = np.stack([res.results[i]["out"] for i in range(N_CORES)], axis=0)
    return out, res


def kernel(f, g, W1, b1, W2, b2):
    out, _ = run(f, g, W1, b1, W2, b2)
    return out
